# revision 3
# baseline (speedup 1.0000x reference)
"""ObjectAttentionBlock (dense transformer) Trainium2 Bass kernel.

Full-input contract: kernel(**inputs) takes the unsharded inputs and
returns the unsharded output. Internally the batch dimension (N=8) is
data-parallel across the 8 NeuronCores: core n processes batch element n.

Per-core computation (channels-first layout [C, pixels] everywhere):
  q  = relu(s1*(W1 @ relu(s0*(W0 @ x) + b0)) + b1)          [C, HW]
  key= relu(s3*(W3 @ relu(s2*(W2 @ p) + b2)) + b3)          [C, K]
  val= relu(s5*(W5 @ relu(s4*(W4 @ p) + b4)) + b5)          [C, K]
  sim= softmax(q^T key / sqrt(C), axis=K)                    [HW, K]
  ctx= (sim @ val^T)^T                                       [C, HW]
  out= relu(s6*(W6 @ ctx) + b6)                              [C, HW]

All matmuls run as float32r (TF32-like single-pass mode, 4x the fp32
matmul rate on the PE). Softmax statistics stay fp32.
"""

import numpy as np

import concourse.bass as bass
import concourse.mybir as mybir
import concourse.tile as tile
from concourse import bacc, bass_utils

N = 8
C = 512
K = 256
H = 128
W_IMG = 128
HW = H * W_IMG
P = 128          # partition width
CT = C // P      # 4 channel tiles
KT = K // P      # 2 region tiles
T = 512          # pixel tile (matmul moving dim / one PSUM bank of fp32)
NT = HW // T     # 32 pixel tiles
ALPHA = 1.0 / float(np.sqrt(C))

f32 = mybir.dt.float32
f32r = mybir.dt.float32r
AF = mybir.ActivationFunctionType


def build_module(n_tiles=NT):
    """Build and compile the per-core Bass module (SPMD: same on all cores)."""
    nc = bacc.Bacc("TRN2", target_bir_lowering=False, debug=False)
    xin = nc.dram_tensor("xin", [C, HW], f32, kind="ExternalInput").ap()
    pin = nc.dram_tensor("pin", [C, K], f32, kind="ExternalInput").ap()
    wt = nc.dram_tensor("wt", [7, C, C], f32, kind="ExternalInput").ap()
    sbc_d = nc.dram_tensor("sbc", [P, 56], f32, kind="ExternalInput").ap()
    idn_d = nc.dram_tensor("idn", [P, P], f32, kind="ExternalInput").ap()
    out_d = nc.dram_tensor("out", [C, HW], f32, kind="ExternalOutput").ap()

    with tile.TileContext(nc) as tc:
        with (
            tc.tile_pool(name="const", bufs=1) as cpool,
            tc.tile_pool(name="loop", bufs=2) as lpool,
            tc.tile_pool(name="small", bufs=2) as smpool,
            tc.tile_pool(name="psc", bufs=4, space="PSUM") as pscp,
            tc.tile_pool(name="pss", bufs=2, space="PSUM") as pssp,
            tc.tile_pool(name="pst", bufs=2, space="PSUM") as pstp,
        ):
            # ---- constants ----
            sbc = cpool.tile([P, 56], f32, name="sbc_t")
            nc.sync.dma_start(sbc[:], sbc_d[:])
            idn = cpool.tile([P, P], f32, name="idn_t")
            nc.sync.dma_start(idn[:].bitcast(f32r), idn_d[:].bitcast(f32r))
            w = [
                [cpool.tile([P, C], f32, name=f"w{i}_{c}") for c in range(CT)]
                for i in range(7)
            ]
            for i in range(7):
                for c in range(CT):
                    nc.sync.dma_start(
                        w[i][c][:].bitcast(f32r),
                        wt[i, c * P : (c + 1) * P, :].bitcast(f32r),
                    )

            def scale_ap(i, o):
                return sbc[:, i * 4 + o : i * 4 + o + 1]

            def bias_ap(i, o):
                return sbc[:, 28 + i * 4 + o : 28 + i * 4 + o + 1]

            def conv(inp, wi, outt, ncols, psum_pool, psum_tag, out_f32r=True):
                """outt[o] = relu(s*W[wi]@inp + b); inp/outt: CT tiles [P, ncols]."""
                for o in range(CT):
                    ps = psum_pool.tile(
                        [P, ncols], f32, name=f"ps_{wi}_{o}", tag=psum_tag
                    )
                    for c in range(CT):
                        nc.tensor.matmul(
                            ps[:],
                            w[wi][c][:, o * P : (o + 1) * P].bitcast(f32r),
                            inp[c][:].bitcast(f32r),
                            start=(c == 0),
                            stop=(c == CT - 1),
                        )
                    dst = outt[o][:].bitcast(f32r) if out_f32r else outt[o][:]
                    nc.scalar.activation(
                        dst, ps[:], AF.Relu, bias=bias_ap(wi, o), scale=scale_ap(wi, o)
                    )

            # ---- setup: key / value from proxy ----
            key = [cpool.tile([P, K], f32, name=f"key{c}") for c in range(CT)]
            valT = [cpool.tile([P, C], f32, name=f"valT{k}") for k in range(KT)]
            with tc.tile_pool(name="setup", bufs=1) as spool:
                p_t = [spool.tile([P, K], f32, name=f"p{c}") for c in range(CT)]
                for c in range(CT):
                    nc.sync.dma_start(
                        p_t[c][:].bitcast(f32r),
                        pin[c * P : (c + 1) * P, :].bitcast(f32r),
                    )
                k1 = [spool.tile([P, K], f32, name=f"k1_{c}") for c in range(CT)]
                conv(p_t, 2, k1, K, pssp, "sps")
                conv(k1, 3, key, K, pssp, "sps")
                v1 = [spool.tile([P, K], f32, name=f"v1_{c}") for c in range(CT)]
                conv(p_t, 4, v1, K, pssp, "sps")
                val = [spool.tile([P, K], f32, name=f"val{c}") for c in range(CT)]
                conv(v1, 5, val, K, pssp, "sps")
                for c in range(CT):
                    for k in range(KT):
                        pt = pstp.tile([P, P], f32, name=f"ptv{c}_{k}", tag="tps")
                        nc.tensor.matmul(
                            pt[:].bitcast(f32r),
                            val[c][:, k * P : (k + 1) * P].bitcast(f32r),
                            idn[:].bitcast(f32r),
                            is_transpose=True,
                        )
                        nc.vector.tensor_copy(
                            valT[k][:, c * P : (c + 1) * P].bitcast(f32r), pt[:]
                        )

            # ---- main pipeline over pixel tiles ----
            def stage_a(t):
                xt = [lpool.tile([P, T], f32, name=f"xt{c}", tag=f"xt{c}") for c in range(CT)]
                for c in range(CT):
                    nc.sync.dma_start(
                        xt[c][:].bitcast(f32r),
                        xin[c * P : (c + 1) * P, t * T : (t + 1) * T].bitcast(f32r),
                    )
                t1 = [lpool.tile([P, T], f32, name=f"t1_{c}", tag=f"t1{c}") for c in range(CT)]
                conv(xt, 0, t1, T, pscp, "cps")
                q = [lpool.tile([P, T], f32, name=f"q{c}", tag=f"q{c}") for c in range(CT)]
                conv(t1, 1, q, T, pscp, "cps")
                prob_n = [
                    lpool.tile([P, K], f32, name=f"pn{pc}", tag=f"pn{pc}")
                    for pc in range(T // P)
                ]
                for pc in range(T // P):
                    ps = pssp.tile([P, K], f32, name=f"ps_sim{pc}", tag="sps")
                    for c in range(CT):
                        nc.tensor.matmul(
                            ps[:],
                            q[c][:, pc * P : (pc + 1) * P].bitcast(f32r),
                            key[c][:].bitcast(f32r),
                            start=(c == 0),
                            stop=(c == CT - 1),
                        )
                    mx = smpool.tile([P, 1], f32, name=f"mx{pc}", tag=f"mx{pc}")
                    nc.vector.reduce_max(mx[:], ps[:], axis=mybir.AxisListType.X)
                    nmx = smpool.tile([P, 1], f32, name=f"nmx{pc}", tag=f"nmx{pc}")
                    nc.scalar.mul(nmx[:], mx[:], -ALPHA)
                    prob = lpool.tile([P, K], f32, name=f"pr{pc}", tag=f"pr{pc}")
                    rsum = smpool.tile([P, 1], f32, name=f"rs{pc}", tag=f"rs{pc}")
                    nc.scalar.activation(
                        prob[:],
                        ps[:],
                        AF.Exp,
                        bias=nmx[:],
                        scale=ALPHA,
                        accum_out=rsum[:],
                    )
                    rrec = smpool.tile([P, 1], f32, name=f"rr{pc}", tag=f"rr{pc}")
                    nc.vector.reciprocal(rrec[:], rsum[:])
                    nc.vector.tensor_scalar_mul(
                        prob_n[pc][:].bitcast(f32r), prob[:], rrec[:]
                    )
                return prob_n

            def stage_b(t, prob_n):
                probT = [
                    lpool.tile([P, T], f32, name=f"pT{k}", tag=f"pT{k}")
                    for k in range(KT)
                ]
                for pc in range(T // P):
                    for k in range(KT):
                        pt = pstp.tile([P, P], f32, name=f"ptp{pc}_{k}", tag="tps")
                        nc.tensor.matmul(
                            pt[:].bitcast(f32r),
                            prob_n[pc][:, k * P : (k + 1) * P].bitcast(f32r),
                            idn[:].bitcast(f32r),
                            is_transpose=True,
                        )
                        nc.vector.tensor_copy(
                            probT[k][:, pc * P : (pc + 1) * P].bitcast(f32r), pt[:]
                        )
                ctx = [lpool.tile([P, T], f32, name=f"cx{c}", tag=f"cx{c}") for c in range(CT)]
                for c in range(CT):
                    ps = pscp.tile([P, T], f32, name=f"ps_ctx{c}", tag="cps")
                    for k in range(KT):
                        nc.tensor.matmul(
                            ps[:],
                            valT[k][:, c * P : (c + 1) * P].bitcast(f32r),
                            probT[k][:].bitcast(f32r),
                            start=(k == 0),
                            stop=(k == KT - 1),
                        )
                    nc.vector.tensor_copy(ctx[c][:].bitcast(f32r), ps[:])
                outt = [
                    lpool.tile([P, T], f32, name=f"ot{o}", tag=f"ot{o}") for o in range(CT)
                ]
                conv(ctx, 6, outt, T, pscp, "cps", out_f32r=False)
                for o in range(CT):
                    nc.sync.dma_start(
                        out_d[o * P : (o + 1) * P, t * T : (t + 1) * T], outt[o][:]
                    )

            prev = None
            for t in range(n_tiles):
                pn = stage_a(t)
                if prev is not None:
                    stage_b(prev[0], prev[1])
                prev = (t, pn)
            stage_b(prev[0], prev[1])

    nc.compile()
    return nc


def make_in_maps(x, proxy, W, s, b):
    wt = np.ascontiguousarray(W.transpose(0, 2, 1)).astype(np.float32)
    sbc = np.concatenate(
        [
            s.reshape(7, CT, P).transpose(2, 0, 1).reshape(P, 7 * CT),
            b.reshape(7, CT, P).transpose(2, 0, 1).reshape(P, 7 * CT),
        ],
        axis=1,
    ).astype(np.float32)
    sbc = np.ascontiguousarray(sbc)
    idn = np.eye(P, dtype=np.float32)
    in_maps = []
    for n in range(N):
        in_maps.append(
            {
                "xin": np.ascontiguousarray(x[n].reshape(C, HW), dtype=np.float32),
                "pin": np.ascontiguousarray(proxy[n].reshape(C, K), dtype=np.float32),
                "wt": wt,
                "sbc": sbc,
                "idn": idn,
            }
        )
    return in_maps


_CACHED = {}


def _get_module():
    if "nc" not in _CACHED:
        _CACHED["nc"] = build_module()
    return _CACHED["nc"]


def kernel(x, proxy, W, s, b):
    nc = _get_module()
    in_maps = make_in_maps(x, proxy, W, s, b)
    res = bass_utils.run_bass_kernel_spmd(nc, in_maps, core_ids=list(range(N)))
    out = np.stack([res.results[n]["out"].reshape(C, H, W_IMG) for n in range(N)])
    return out.astype(np.float32)


# revision 4
# speedup vs baseline: 1.0378x; 1.0378x over previous
"""ObjectAttentionBlock (dense transformer) Trainium2 Bass kernel.

Full-input contract: kernel(**inputs) takes the unsharded inputs and
returns the unsharded output. Internally the batch dimension (N=8) is
data-parallel across the 8 NeuronCores: core n processes batch element n.

Per-core computation (channels-first layout [C, pixels] everywhere):
  q  = relu(s1*(W1 @ relu(s0*(W0 @ x) + b0)) + b1)          [C, HW]
  key= relu(s3*(W3 @ relu(s2*(W2 @ p) + b2)) + b3)          [C, K]
  val= relu(s5*(W5 @ relu(s4*(W4 @ p) + b4)) + b5)          [C, K]
  sim= softmax(q^T key / sqrt(C), axis=K)                    [HW, K]
  ctx= (sim @ val^T)^T                                       [C, HW]
  out= relu(s6*(W6 @ ctx) + b6)                              [C, HW]

All matmuls run as float32r (TF32-like single-pass mode, 4x the fp32
matmul rate on the PE). Softmax statistics stay fp32.
"""

import numpy as np

import concourse.bass as bass
import concourse.mybir as mybir
import concourse.tile as tile
from concourse import bacc, bass_utils

N = 8
C = 512
K = 256
H = 128
W_IMG = 128
HW = H * W_IMG
P = 128          # partition width
CT = C // P      # 4 channel tiles
KT = K // P      # 2 region tiles
T = 512          # pixel tile (matmul moving dim / one PSUM bank of fp32)
NT = HW // T     # 32 pixel tiles
ALPHA = 1.0 / float(np.sqrt(C))

f32 = mybir.dt.float32
f32r = mybir.dt.float32r
AF = mybir.ActivationFunctionType


def build_module(n_tiles=NT):
    """Build and compile the per-core Bass module (SPMD: same on all cores)."""
    nc = bacc.Bacc("TRN2", target_bir_lowering=False, debug=False)
    xin = nc.dram_tensor("xin", [C, HW], f32, kind="ExternalInput").ap()
    pin = nc.dram_tensor("pin", [C, K], f32, kind="ExternalInput").ap()
    wt = nc.dram_tensor("wt", [7, C, C], f32, kind="ExternalInput").ap()
    sbc_d = nc.dram_tensor("sbc", [P, 56], f32, kind="ExternalInput").ap()
    idn_d = nc.dram_tensor("idn", [P, P], f32, kind="ExternalInput").ap()
    out_d = nc.dram_tensor("out", [C, HW], f32, kind="ExternalOutput").ap()

    with tile.TileContext(nc) as tc:
        with (
            tc.tile_pool(name="const", bufs=1) as cpool,
            tc.tile_pool(name="loop", bufs=2) as lpool,
            tc.tile_pool(name="small", bufs=2) as smpool,
            tc.tile_pool(name="psc", bufs=4, space="PSUM") as pscp,
            tc.tile_pool(name="pss", bufs=2, space="PSUM") as pssp,
            tc.tile_pool(name="ptp", bufs=2, space="PSUM") as ptp,
        ):
            # ---- constants ----
            sbc = cpool.tile([P, 56], f32, name="sbc_t")
            nc.sync.dma_start(sbc[:], sbc_d[:])
            idn = cpool.tile([P, P], f32, name="idn_t")
            nc.sync.dma_start(idn[:].bitcast(f32r), idn_d[:].bitcast(f32r))
            w = [
                [cpool.tile([P, C], f32, name=f"w{i}_{c}") for c in range(CT)]
                for i in range(7)
            ]
            p_t = [cpool.tile([P, K], f32, name=f"p{c}") for c in range(CT)]
            for c in range(CT):
                nc.sync.dma_start(
                    p_t[c][:].bitcast(f32r),
                    pin[c * P : (c + 1) * P, :].bitcast(f32r),
                )
            for i in (2, 4, 3, 5, 0, 1, 6):
                for c in range(CT):
                    nc.sync.dma_start(
                        w[i][c][:].bitcast(f32r),
                        wt[i, c * P : (c + 1) * P, :].bitcast(f32r),
                    )

            def scale_ap(i, o):
                return sbc[:, i * 4 + o : i * 4 + o + 1]

            def bias_ap(i, o):
                return sbc[:, 28 + i * 4 + o : 28 + i * 4 + o + 1]

            def conv(inp, wi, outt, ncols, psum_pool, psum_tag, out_f32r=True):
                """outt[o] = relu(s*W[wi]@inp + b); inp/outt: CT tiles [P, ncols]."""
                for o in range(CT):
                    ps = psum_pool.tile(
                        [P, ncols], f32, name=f"ps_{wi}_{o}", tag=psum_tag
                    )
                    for c in range(CT):
                        nc.tensor.matmul(
                            ps[:],
                            w[wi][c][:, o * P : (o + 1) * P].bitcast(f32r),
                            inp[c][:].bitcast(f32r),
                            start=(c == 0),
                            stop=(c == CT - 1),
                        )
                    dst = outt[o][:].bitcast(f32r) if out_f32r else outt[o][:]
                    nc.scalar.activation(
                        dst, ps[:], AF.Relu, bias=bias_ap(wi, o), scale=scale_ap(wi, o)
                    )

            # ---- setup: key / value from proxy ----
            key = [cpool.tile([P, K], f32, name=f"key{c}") for c in range(CT)]
            valT = [cpool.tile([P, C], f32, name=f"valT{k}") for k in range(KT)]
            with tc.tile_pool(name="setup", bufs=1) as spool:
                k1 = [spool.tile([P, K], f32, name=f"k1_{c}") for c in range(CT)]
                conv(p_t, 2, k1, K, pssp, "sps")
                conv(k1, 3, key, K, pssp, "sps")
                v1 = [spool.tile([P, K], f32, name=f"v1_{c}") for c in range(CT)]
                conv(p_t, 4, v1, K, pssp, "sps")
                val = [spool.tile([P, K], f32, name=f"val{c}") for c in range(CT)]
                conv(v1, 5, val, K, pssp, "sps")
                for k in range(KT):
                    pt = ptp.tile([P, C], f32, name=f"ptv{k}", tag="ptp")
                    for c in range(CT):
                        nc.tensor.matmul(
                            pt[:, c * P : (c + 1) * P].bitcast(f32r),
                            val[c][:, k * P : (k + 1) * P].bitcast(f32r),
                            idn[:].bitcast(f32r),
                            is_transpose=True,
                        )
                    nc.vector.tensor_copy(valT[k][:].bitcast(f32r), pt[:])

            # ---- main pipeline over pixel tiles ----
            def stage_a(t):
                xt = [lpool.tile([P, T], f32, name=f"xt{c}", tag=f"xt{c}") for c in range(CT)]
                for c in range(CT):
                    nc.sync.dma_start(
                        xt[c][:].bitcast(f32r),
                        xin[c * P : (c + 1) * P, t * T : (t + 1) * T].bitcast(f32r),
                    )
                t1 = [lpool.tile([P, T], f32, name=f"t1_{c}", tag=f"t1{c}") for c in range(CT)]
                conv(xt, 0, t1, T, pscp, "cps")
                q = [lpool.tile([P, T], f32, name=f"q{c}", tag=f"q{c}") for c in range(CT)]
                conv(t1, 1, q, T, pscp, "cps")
                prob_n = [
                    lpool.tile([P, K], f32, name=f"pn{pc}", tag=f"pn{pc}")
                    for pc in range(T // P)
                ]
                for pc in range(T // P):
                    ps = pssp.tile([P, K], f32, name=f"ps_sim{pc}", tag="sps")
                    for c in range(CT):
                        nc.tensor.matmul(
                            ps[:],
                            q[c][:, pc * P : (pc + 1) * P].bitcast(f32r),
                            key[c][:].bitcast(f32r),
                            start=(c == 0),
                            stop=(c == CT - 1),
                        )
                    mx = smpool.tile([P, 1], f32, name=f"mx{pc}", tag=f"mx{pc}")
                    nc.vector.reduce_max(mx[:], ps[:], axis=mybir.AxisListType.X)
                    nmx = smpool.tile([P, 1], f32, name=f"nmx{pc}", tag=f"nmx{pc}")
                    nc.scalar.mul(nmx[:], mx[:], -ALPHA)
                    prob = lpool.tile([P, K], f32, name=f"pr{pc}", tag=f"pr{pc}")
                    rsum = smpool.tile([P, 1], f32, name=f"rs{pc}", tag=f"rs{pc}")
                    nc.scalar.activation(
                        prob[:],
                        ps[:],
                        AF.Exp,
                        bias=nmx[:],
                        scale=ALPHA,
                        accum_out=rsum[:],
                    )
                    rrec = smpool.tile([P, 1], f32, name=f"rr{pc}", tag=f"rr{pc}")
                    nc.vector.reciprocal(rrec[:], rsum[:])
                    nc.vector.tensor_scalar_mul(
                        prob_n[pc][:].bitcast(f32r), prob[:], rrec[:]
                    )
                return prob_n

            def stage_b(t, prob_n):
                probT = [
                    lpool.tile([P, T], f32, name=f"pT{k}", tag=f"pT{k}")
                    for k in range(KT)
                ]
                for k in range(KT):
                    pt = ptp.tile([P, T], f32, name=f"ptp{k}", tag="ptp")
                    for pc in range(T // P):
                        nc.tensor.matmul(
                            pt[:, pc * P : (pc + 1) * P].bitcast(f32r),
                            prob_n[pc][:, k * P : (k + 1) * P].bitcast(f32r),
                            idn[:].bitcast(f32r),
                            is_transpose=True,
                        )
                    nc.vector.tensor_copy(probT[k][:].bitcast(f32r), pt[:])
                ctx = [lpool.tile([P, T], f32, name=f"cx{c}", tag=f"cx{c}") for c in range(CT)]
                for c in range(CT):
                    ps = pscp.tile([P, T], f32, name=f"ps_ctx{c}", tag="cps")
                    for k in range(KT):
                        nc.tensor.matmul(
                            ps[:],
                            valT[k][:, c * P : (c + 1) * P].bitcast(f32r),
                            probT[k][:].bitcast(f32r),
                            start=(k == 0),
                            stop=(k == KT - 1),
                        )
                    nc.vector.tensor_copy(ctx[c][:].bitcast(f32r), ps[:])
                outt = [
                    lpool.tile([P, T], f32, name=f"ot{o}", tag=f"ot{o}") for o in range(CT)
                ]
                conv(ctx, 6, outt, T, pscp, "cps", out_f32r=False)
                for o in range(CT):
                    nc.sync.dma_start(
                        out_d[o * P : (o + 1) * P, t * T : (t + 1) * T], outt[o][:]
                    )

            prev = None
            for t in range(n_tiles):
                pn = stage_a(t)
                if prev is not None:
                    stage_b(prev[0], prev[1])
                prev = (t, pn)
            stage_b(prev[0], prev[1])

    nc.compile()
    return nc


def make_in_maps(x, proxy, W, s, b):
    wt = np.ascontiguousarray(W.transpose(0, 2, 1)).astype(np.float32)
    sbc = np.concatenate(
        [
            s.reshape(7, CT, P).transpose(2, 0, 1).reshape(P, 7 * CT),
            b.reshape(7, CT, P).transpose(2, 0, 1).reshape(P, 7 * CT),
        ],
        axis=1,
    ).astype(np.float32)
    sbc = np.ascontiguousarray(sbc)
    idn = np.eye(P, dtype=np.float32)
    in_maps = []
    for n in range(N):
        in_maps.append(
            {
                "xin": np.ascontiguousarray(x[n].reshape(C, HW), dtype=np.float32),
                "pin": np.ascontiguousarray(proxy[n].reshape(C, K), dtype=np.float32),
                "wt": wt,
                "sbc": sbc,
                "idn": idn,
            }
        )
    return in_maps


_CACHED = {}


def _get_module():
    if "nc" not in _CACHED:
        _CACHED["nc"] = build_module()
    return _CACHED["nc"]


def kernel(x, proxy, W, s, b):
    nc = _get_module()
    in_maps = make_in_maps(x, proxy, W, s, b)
    res = bass_utils.run_bass_kernel_spmd(nc, in_maps, core_ids=list(range(N)))
    out = np.stack([res.results[n]["out"].reshape(C, H, W_IMG) for n in range(N)])
    return out.astype(np.float32)


# revision 5
# speedup vs baseline: 1.1026x; 1.0625x over previous
"""ObjectAttentionBlock (dense transformer) Trainium2 Bass kernel.

Full-input contract: kernel(**inputs) takes the unsharded inputs and
returns the unsharded output. Internally the batch dimension (N=8) is
data-parallel across the 8 NeuronCores: core n processes batch element n.

Per-core computation (channels-first layout [C, pixels] everywhere):
  q  = relu(s1*(W1 @ relu(s0*(W0 @ x) + b0)) + b1)          [C, HW]
  key= relu(s3*(W3 @ relu(s2*(W2 @ p) + b2)) + b3)          [C, K]
  val= relu(s5*(W5 @ relu(s4*(W4 @ p) + b4)) + b5)          [C, K]
  sim= softmax(q^T key / sqrt(C), axis=K)                    [HW, K]
  ctx= (sim @ val^T)^T                                       [C, HW]
  out= relu(s6*(W6 @ ctx) + b6)                              [C, HW]

All matmuls run as float32r (TF32-like single-pass mode, 4x the fp32
matmul rate on the PE). Softmax statistics stay fp32.
"""

import numpy as np

import concourse.bass as bass
import concourse.mybir as mybir
import concourse.tile as tile
from concourse import bacc, bass_utils

N = 8
C = 512
K = 256
H = 128
W_IMG = 128
HW = H * W_IMG
P = 128          # partition width
CT = C // P      # 4 channel tiles
KT = K // P      # 2 region tiles
T = 512          # pixel tile (matmul moving dim / one PSUM bank of fp32)
NT = HW // T     # 32 pixel tiles
ALPHA = 1.0 / float(np.sqrt(C))

f32 = mybir.dt.float32
f32r = mybir.dt.float32r
AF = mybir.ActivationFunctionType


def build_module(n_tiles=NT):
    """Build and compile the per-core Bass module (SPMD: same on all cores)."""
    nc = bacc.Bacc("TRN2", target_bir_lowering=False, debug=False)
    xin = nc.dram_tensor("xin", [C, HW], f32, kind="ExternalInput").ap()
    pin = nc.dram_tensor("pin", [C, K], f32, kind="ExternalInput").ap()
    wt = nc.dram_tensor("wt", [7, C, C], f32, kind="ExternalInput").ap()
    sbc_d = nc.dram_tensor("sbc", [P, 56], f32, kind="ExternalInput").ap()
    idn_d = nc.dram_tensor("idn", [P, P], f32, kind="ExternalInput").ap()
    out_d = nc.dram_tensor("out", [C, HW], f32, kind="ExternalOutput").ap()

    with tile.TileContext(nc) as tc:
        with (
            tc.tile_pool(name="const", bufs=1) as cpool,
            tc.tile_pool(name="loop", bufs=2) as lpool,
            tc.tile_pool(name="small", bufs=2) as smpool,
            tc.tile_pool(name="psc", bufs=5, space="PSUM") as pscp,
            tc.tile_pool(name="pss", bufs=2, space="PSUM") as pssp,
            tc.tile_pool(name="ptp", bufs=1, space="PSUM") as ptp,
        ):
            # ---- constants ----
            sbc = cpool.tile([P, 56], f32, name="sbc_t")
            nc.sync.dma_start(sbc[:], sbc_d[:])
            idn = cpool.tile([P, P], f32, name="idn_t")
            nc.sync.dma_start(idn[:].bitcast(f32r), idn_d[:].bitcast(f32r))
            w = [
                [cpool.tile([P, C], f32, name=f"w{i}_{c}") for c in range(CT)]
                for i in range(7)
            ]
            p_t = [cpool.tile([P, K], f32, name=f"p{c}") for c in range(CT)]
            for c in range(CT):
                nc.sync.dma_start(
                    p_t[c][:].bitcast(f32r),
                    pin[c * P : (c + 1) * P, :].bitcast(f32r),
                )
            for i in (2, 4, 3, 5, 0, 1, 6):
                for c in range(CT):
                    nc.sync.dma_start(
                        w[i][c][:].bitcast(f32r),
                        wt[i, c * P : (c + 1) * P, :].bitcast(f32r),
                    )

            def scale_ap(i, o):
                return sbc[:, i * 4 + o : i * 4 + o + 1]

            def bias_ap(i, o):
                return sbc[:, 28 + i * 4 + o : 28 + i * 4 + o + 1]

            def conv(inp, wi, outt, ncols, psum_pool, psum_tag, out_f32r=True):
                """outt[o] = relu(s*W[wi]@inp + b); inp/outt: CT tiles [P, ncols]."""
                for o in range(CT):
                    ps = psum_pool.tile(
                        [P, ncols], f32, name=f"ps_{wi}_{o}", tag=psum_tag
                    )
                    for c in range(CT):
                        nc.tensor.matmul(
                            ps[:],
                            w[wi][c][:, o * P : (o + 1) * P].bitcast(f32r),
                            inp[c][:].bitcast(f32r),
                            start=(c == 0),
                            stop=(c == CT - 1),
                        )
                    dst = outt[o][:].bitcast(f32r) if out_f32r else outt[o][:]
                    nc.scalar.activation(
                        dst, ps[:], AF.Relu, bias=bias_ap(wi, o), scale=scale_ap(wi, o)
                    )

            # ---- setup: key / value from proxy ----
            key = [cpool.tile([P, K], f32, name=f"key{c}") for c in range(CT)]
            valT = [cpool.tile([P, C], f32, name=f"valT{k}") for k in range(KT)]
            with tc.tile_pool(name="setup", bufs=1) as spool:
                k1 = [spool.tile([P, K], f32, name=f"k1_{c}") for c in range(CT)]
                conv(p_t, 2, k1, K, pssp, "sps")
                conv(k1, 3, key, K, pssp, "sps")
                v1 = [spool.tile([P, K], f32, name=f"v1_{c}") for c in range(CT)]
                conv(p_t, 4, v1, K, pssp, "sps")
                val = [spool.tile([P, K], f32, name=f"val{c}") for c in range(CT)]
                conv(v1, 5, val, K, pssp, "sps")
                for k in range(KT):
                    pt = ptp.tile([P, C], f32, name=f"ptv{k}", tag="ptp")
                    for c in range(CT):
                        nc.tensor.matmul(
                            pt[:, c * P : (c + 1) * P].bitcast(f32r),
                            val[c][:, k * P : (k + 1) * P].bitcast(f32r),
                            idn[:].bitcast(f32r),
                            is_transpose=True,
                        )
                    nc.vector.tensor_copy(valT[k][:].bitcast(f32r), pt[:])

            # ---- main pipeline over pixel tiles ----
            def stage_a(t):
                xt = [lpool.tile([P, T], f32, name=f"xt{c}", tag=f"xt{c}") for c in range(CT)]
                for c in range(CT):
                    nc.sync.dma_start(
                        xt[c][:].bitcast(f32r),
                        xin[c * P : (c + 1) * P, t * T : (t + 1) * T].bitcast(f32r),
                    )
                t1 = [lpool.tile([P, T], f32, name=f"t1_{c}", tag=f"t1{c}") for c in range(CT)]
                conv(xt, 0, t1, T, pscp, "cps")
                q = [lpool.tile([P, T], f32, name=f"q{c}", tag=f"q{c}") for c in range(CT)]
                conv(t1, 1, q, T, pscp, "cps")
                prob_n = [
                    lpool.tile([P, K], f32, name=f"pn{pc}", tag=f"pn{pc}")
                    for pc in range(T // P)
                ]
                for pc in range(T // P):
                    ps = pssp.tile([P, K], f32, name=f"ps_sim{pc}", tag="sps")
                    for c in range(CT):
                        nc.tensor.matmul(
                            ps[:],
                            q[c][:, pc * P : (pc + 1) * P].bitcast(f32r),
                            key[c][:].bitcast(f32r),
                            start=(c == 0),
                            stop=(c == CT - 1),
                        )
                    mx = smpool.tile([P, 1], f32, name=f"mx{pc}", tag=f"mx{pc}")
                    nc.vector.reduce_max(mx[:], ps[:], axis=mybir.AxisListType.X)
                    nmx = smpool.tile([P, 1], f32, name=f"nmx{pc}", tag=f"nmx{pc}")
                    nc.scalar.mul(nmx[:], mx[:], -ALPHA)
                    prob = lpool.tile([P, K], f32, name=f"pr{pc}", tag=f"pr{pc}")
                    rsum = smpool.tile([P, 1], f32, name=f"rs{pc}", tag=f"rs{pc}")
                    nc.scalar.activation(
                        prob[:],
                        ps[:],
                        AF.Exp,
                        bias=nmx[:],
                        scale=ALPHA,
                        accum_out=rsum[:],
                    )
                    rrec = smpool.tile([P, 1], f32, name=f"rr{pc}", tag=f"rr{pc}")
                    nc.vector.reciprocal(rrec[:], rsum[:])
                    nc.vector.tensor_scalar_mul(
                        prob_n[pc][:].bitcast(f32r), prob[:], rrec[:]
                    )
                return prob_n

            def stage_b(t, prob_n):
                probT = [
                    lpool.tile([P, T], f32, name=f"pT{k}", tag=f"pT{k}")
                    for k in range(KT)
                ]
                for k in range(KT):
                    pt = ptp.tile([P, T], f32, name=f"ptp{k}", tag="ptp")
                    for pc in range(T // P):
                        nc.tensor.matmul(
                            pt[:, pc * P : (pc + 1) * P].bitcast(f32r),
                            prob_n[pc][:, k * P : (k + 1) * P].bitcast(f32r),
                            idn[:].bitcast(f32r),
                            is_transpose=True,
                        )
                    nc.vector.tensor_copy(probT[k][:].bitcast(f32r), pt[:])
                ctx = [lpool.tile([P, T], f32, name=f"cx{c}", tag=f"cx{c}") for c in range(CT)]
                for c in range(CT):
                    ps = pscp.tile([P, T], f32, name=f"ps_ctx{c}", tag="cps")
                    for k in range(KT):
                        nc.tensor.matmul(
                            ps[:],
                            valT[k][:, c * P : (c + 1) * P].bitcast(f32r),
                            probT[k][:].bitcast(f32r),
                            start=(k == 0),
                            stop=(k == KT - 1),
                        )
                    nc.vector.tensor_copy(ctx[c][:].bitcast(f32r), ps[:])
                outt = [
                    lpool.tile([P, T], f32, name=f"ot{o}", tag=f"ot{o}") for o in range(CT)
                ]
                conv(ctx, 6, outt, T, pscp, "cps", out_f32r=False)
                for o in range(CT):
                    nc.sync.dma_start(
                        out_d[o * P : (o + 1) * P, t * T : (t + 1) * T], outt[o][:]
                    )

            prev = None
            for t in range(n_tiles):
                pn = stage_a(t)
                if prev is not None:
                    stage_b(prev[0], prev[1])
                prev = (t, pn)
            stage_b(prev[0], prev[1])

    nc.compile()
    return nc


def make_in_maps(x, proxy, W, s, b):
    wt = np.ascontiguousarray(W.transpose(0, 2, 1)).astype(np.float32)
    sbc = np.concatenate(
        [
            s.reshape(7, CT, P).transpose(2, 0, 1).reshape(P, 7 * CT),
            b.reshape(7, CT, P).transpose(2, 0, 1).reshape(P, 7 * CT),
        ],
        axis=1,
    ).astype(np.float32)
    sbc = np.ascontiguousarray(sbc)
    idn = np.eye(P, dtype=np.float32)
    in_maps = []
    for n in range(N):
        in_maps.append(
            {
                "xin": np.ascontiguousarray(x[n].reshape(C, HW), dtype=np.float32),
                "pin": np.ascontiguousarray(proxy[n].reshape(C, K), dtype=np.float32),
                "wt": wt,
                "sbc": sbc,
                "idn": idn,
            }
        )
    return in_maps


_CACHED = {}


def _get_module():
    if "nc" not in _CACHED:
        _CACHED["nc"] = build_module()
    return _CACHED["nc"]


def kernel(x, proxy, W, s, b):
    nc = _get_module()
    in_maps = make_in_maps(x, proxy, W, s, b)
    res = bass_utils.run_bass_kernel_spmd(nc, in_maps, core_ids=list(range(N)))
    out = np.stack([res.results[n]["out"].reshape(C, H, W_IMG) for n in range(N)])
    return out.astype(np.float32)
